# revision 4
# baseline (speedup 1.0000x reference)
"""NT-Xent (SimCLR) contrastive loss on 8 Trainium2 NeuronCores.

Sharding: data-parallel over the 8192 rows of the similarity matrix.
Core c computes rows [c*1024, (c+1)*1024): row-sharded sim = zn_rows @ zn.T,
fused exp+rowsum on the scalar engine, positives via an elementwise
zrT*zpT product-sum on the vector engine, per-core partial loss sum;
host sums the 8 scalars and divides by 8192.
"""

import sys

import numpy as np

sys.path.insert(0, "/opt/trn_rl_repo")

import ml_dtypes  # noqa: E402

B = 4096
D = 256
N2 = 2 * B            # 8192 rows/cols of sim
NCORES = 8
RPC = N2 // NCORES    # 1024 rows per core
TEMPERATURE = 0.5
INV_T = 1.0 / TEMPERATURE
EPS = 1e-8
KCH = D // 128        # 2 contraction chunks of 128
MCH = RPC // 128      # 8 row chunks per core
CG = 4                # column groups
CGW = N2 // CG        # 2048 columns per group
NSUB = CGW // 512     # 4 matmul N-subtiles per group

_compiled_nc = None


def _build_nc():
    from concourse import bacc, mybir, tile

    f32 = mybir.dt.float32
    bf16 = mybir.dt.bfloat16
    nc = bacc.Bacc(
        "TRN2", target_bir_lowering=False, debug=False, num_devices=NCORES
    )

    znT_d = nc.dram_tensor("znT", [KCH, 128, N2], bf16, kind="ExternalInput")
    zrT_d = nc.dram_tensor("zrowsT", [KCH, 128, RPC], bf16, kind="ExternalInput")
    zpT_d = nc.dram_tensor("zposT", [KCH, 128, RPC], bf16, kind="ExternalInput")
    out_d = nc.dram_tensor("out", [1, 1], f32, kind="ExternalOutput")
    ones_d = nc.inline_tensor(np.ones((128, 1), dtype=np.float32), name="ones")

    # Our row sums include the self-similarity term exp(<z,z>/T) ~= exp(2)
    # which the reference masks out; subtract it before the log.
    exp_self = float(np.exp(INV_T))

    with tile.TileContext(nc) as tc:
        with (
            tc.tile_pool(name="sb", bufs=1) as sb,
            tc.tile_pool(name="ps", bufs=2, space="PSUM") as ps,
        ):
            znT = sb.tile([128, KCH, N2], bf16)
            zrT = sb.tile([128, KCH, RPC], bf16)
            zpT = sb.tile([128, KCH, RPC], bf16)
            ones = sb.tile([128, 1], f32)
            denom = sb.tile([128, MCH, CG], f32)
            lnbias = sb.tile([128, 1], f32)
            nc.gpsimd.memset(lnbias[:], -exp_self)

            for k in range(KCH):
                nc.sync.dma_start(zrT[:, k, :], zrT_d[k])
                nc.sync.dma_start(zpT[:, k, :], zpT_d[k])
            nc.sync.dma_start(ones[:], ones_d[:])
            for g in range(CG):
                for k in range(KCH):
                    nc.sync.dma_start(
                        znT[:, k, g * CGW:(g + 1) * CGW],
                        znT_d[k, :, g * CGW:(g + 1) * CGW],
                    )

            # Main loop: row-chunk m (128 rows) x column-group g (2048 cols).
            for m in range(MCH):
                for g in range(CG):
                    pt = ps.tile([128, CGW], f32, name="pt", tag="pt")
                    for k in range(KCH):
                        lhsT = zrT[:, k, m * 128:(m + 1) * 128]
                        for s in range(NSUB):
                            c0 = g * CGW + s * 512
                            nc.tensor.matmul(
                                pt[:, s * 512:(s + 1) * 512],
                                lhsT,
                                znT[:, k, c0:c0 + 512],
                                start=(k == 0),
                                stop=(k == KCH - 1),
                            )
                    # exp(sim) in place + per-row partial sum (free-dim accum)
                    nc.scalar.activation(
                        pt[:],
                        pt[:],
                        mybir.ActivationFunctionType.Exp,
                        scale=INV_T,
                        accum_out=denom[:, m, g:g + 1],
                    )

            # Positives: sum_i <zr_i, zp_i>/T = full sum of (zrT*INV_T)*zpT.
            # (Partition k is the contraction dim, so the per-partition accum
            # plus the final cross-partition matmul completes the dot sums.)
            prod = sb.tile([128, KCH, RPC], f32)
            possum = sb.tile([128, 1], f32)
            nc.vector.scalar_tensor_tensor(
                out=prod[:],
                in0=zrT[:],
                scalar=INV_T,
                in1=zpT[:],
                op0=mybir.AluOpType.mult,
                op1=mybir.AluOpType.mult,
                accum_out=possum[:],
            )

            denr = sb.tile([128, MCH], f32)
            nc.vector.tensor_reduce(
                denr[:], denom[:], axis=mybir.AxisListType.X,
                op=mybir.AluOpType.add,
            )
            logden = sb.tile([128, MCH], f32)
            ldsum = sb.tile([128, 1], f32)
            nc.scalar.activation(
                logden[:],
                denr[:],
                mybir.ActivationFunctionType.Ln,
                bias=lnbias[:],
                accum_out=ldsum[:],
            )
            rowsum = sb.tile([128, 1], f32)
            nc.vector.tensor_tensor(
                out=rowsum[:],
                in0=ldsum[:],
                in1=possum[:],
                op=mybir.AluOpType.subtract,
            )
            # cross-partition sum via ones-matmul
            pt = ps.tile([128, CGW], f32, name="pt", tag="pt")
            nc.tensor.matmul(pt[:1, :1], ones[:], rowsum[:], start=True, stop=True)
            outsb = sb.tile([1, 1], f32)
            nc.vector.tensor_copy(outsb[:], pt[:1, :1])
            nc.sync.dma_start(out_d[:], outsb[:])

    nc.compile()
    return nc


def _prep_in_maps(z_i: np.ndarray, z_j: np.ndarray) -> list[dict]:
    z = np.concatenate(
        [np.asarray(z_i, dtype=np.float32), np.asarray(z_j, dtype=np.float32)],
        axis=0,
    )
    norms = np.sqrt(np.sum(z * z, axis=1, keepdims=True))
    zn = z / np.maximum(norms, EPS)
    znT = np.ascontiguousarray(zn.T).astype(ml_dtypes.bfloat16)
    znT_k = np.ascontiguousarray(znT.reshape(KCH, 128, N2))
    in_maps = []
    for c in range(NCORES):
        r0 = c * RPC
        p0 = (r0 + B) % N2
        in_maps.append(
            {
                "znT": znT_k,
                "zrowsT": np.ascontiguousarray(znT_k[:, :, r0:r0 + RPC]),
                "zposT": np.ascontiguousarray(znT_k[:, :, p0:p0 + RPC]),
            }
        )
    return in_maps


def _run(in_maps: list[dict], trace: bool = False):
    global _compiled_nc
    if _compiled_nc is None:
        _compiled_nc = _build_nc()
    from concourse.bass_utils import run_bass_kernel_spmd

    return run_bass_kernel_spmd(
        _compiled_nc, in_maps, core_ids=list(range(NCORES)), trace=trace
    )


def kernel(**inputs: np.ndarray) -> np.ndarray:
    in_maps = _prep_in_maps(inputs["z_i"], inputs["z_j"])
    res = _run(in_maps, trace=False)
    total = 0.0
    for r in res.results:
        total += float(r["out"][0, 0])
    return np.array(total / N2, dtype=np.float32)


# revision 5
# speedup vs baseline: 1.0995x; 1.0995x over previous
"""NT-Xent (SimCLR) contrastive loss on 8 Trainium2 NeuronCores.

Sharding: data-parallel over the 8192 rows of the similarity matrix.
Core c computes rows [c*1024, (c+1)*1024): row-sharded sim = zn_rows @ zn.T
as fp8e4 DoubleRow matmuls (host scales zn by 16 so fp8 uses its normal
range; the ACT exp scale folds the 1/256 back out), fused exp+rowsum on
the scalar engine, positives via an elementwise bf16 zrT*zpT product-sum
on the vector engine, per-core partial loss sum; host sums the 8 scalars
and divides by 8192.
"""

import sys

import numpy as np

sys.path.insert(0, "/opt/trn_rl_repo")

import ml_dtypes  # noqa: E402

B = 4096
D = 256
N2 = 2 * B            # 8192 rows/cols of sim
NCORES = 8
RPC = N2 // NCORES    # 1024 rows per core
TEMPERATURE = 0.5
INV_T = 1.0 / TEMPERATURE
EPS = 1e-8
KCH = D // 128        # 2 contraction chunks of 128
MCH = RPC // 128      # 8 row chunks per core
CG = 4                # column groups
CGW = N2 // CG        # 2048 columns per group
NSUB = CGW // 512     # 4 matmul N-subtiles per group
FP8_SCALE = 16.0      # host-side scale before fp8e4 quantization

_compiled_nc = None


def _build_nc():
    from concourse import bacc, mybir, tile

    f32 = mybir.dt.float32
    bf16 = mybir.dt.bfloat16
    fp8 = mybir.dt.float8e4
    nc = bacc.Bacc(
        "TRN2", target_bir_lowering=False, debug=False, num_devices=NCORES
    )

    znT8_d = nc.dram_tensor("znT8", [128, KCH, N2], fp8, kind="ExternalInput")
    zrT8_d = nc.dram_tensor("zrT8", [128, KCH, RPC], fp8, kind="ExternalInput")
    zrT_d = nc.dram_tensor("zrowsT", [KCH, 128, RPC], bf16, kind="ExternalInput")
    zpT_d = nc.dram_tensor("zposT", [KCH, 128, RPC], bf16, kind="ExternalInput")
    out_d = nc.dram_tensor("out", [1, 1], f32, kind="ExternalOutput")
    ones_d = nc.inline_tensor(np.ones((128, 1), dtype=np.float32), name="ones")

    # Our row sums include the self-similarity term exp(<z,z>/T) ~= exp(2)
    # which the reference masks out; subtract it before the log.
    exp_self = float(np.exp(INV_T))

    with tile.TileContext(nc) as tc:
        with (
            tc.tile_pool(name="sb", bufs=1) as sb,
            tc.tile_pool(name="ps", bufs=2, space="PSUM") as ps,
        ):
            znT8 = sb.tile([128, KCH, N2], fp8)
            zrT8 = sb.tile([128, KCH, RPC], fp8)
            zrT = sb.tile([128, KCH, RPC], bf16)
            zpT = sb.tile([128, KCH, RPC], bf16)
            ones = sb.tile([128, 1], f32)
            denom = sb.tile([128, MCH, CG], f32)
            lnbias = sb.tile([128, 1], f32)
            nc.gpsimd.memset(lnbias[:], -exp_self)

            nc.sync.dma_start(zrT8[:], zrT8_d[:])
            for g in range(CG):
                nc.sync.dma_start(
                    znT8[:, :, g * CGW:(g + 1) * CGW],
                    znT8_d[:, :, g * CGW:(g + 1) * CGW],
                )
            for k in range(KCH):
                nc.sync.dma_start(zrT[:, k, :], zrT_d[k])
                nc.sync.dma_start(zpT[:, k, :], zpT_d[k])
            nc.sync.dma_start(ones[:], ones_d[:])

            # Main loop: row-chunk m (128 rows) x column-group g (2048 cols).
            # DoubleRow fp8: one matmul covers the full 256-deep contraction.
            for m in range(MCH):
                lhsT = zrT8[:, :, m * 128:(m + 1) * 128]
                for g in range(CG):
                    pt = ps.tile([128, CGW], f32, name="pt", tag="pt")
                    for s in range(NSUB):
                        c0 = g * CGW + s * 512
                        nc.tensor.matmul(
                            pt[:, s * 512:(s + 1) * 512],
                            lhsT,
                            znT8[:, :, c0:c0 + 512],
                            start=True,
                            stop=True,
                            perf_mode=mybir.MatmulPerfMode.DoubleRow,
                        )
                    # exp(sim) in place + per-row partial sum (free-dim accum)
                    nc.scalar.activation(
                        pt[:],
                        pt[:],
                        mybir.ActivationFunctionType.Exp,
                        scale=INV_T / (FP8_SCALE * FP8_SCALE),
                        accum_out=denom[:, m, g:g + 1],
                    )

            # Positives: sum_i <zr_i, zp_i>/T = full sum of (zrT*INV_T)*zpT.
            # (Partition k is the contraction dim, so the per-partition accum
            # plus the final cross-partition matmul completes the dot sums.)
            prod = sb.tile([128, KCH, RPC], f32)
            possum = sb.tile([128, 1], f32)
            nc.vector.scalar_tensor_tensor(
                out=prod[:],
                in0=zrT[:],
                scalar=INV_T,
                in1=zpT[:],
                op0=mybir.AluOpType.mult,
                op1=mybir.AluOpType.mult,
                accum_out=possum[:],
            )

            denr = sb.tile([128, MCH], f32)
            nc.vector.tensor_reduce(
                denr[:], denom[:], axis=mybir.AxisListType.X,
                op=mybir.AluOpType.add,
            )
            logden = sb.tile([128, MCH], f32)
            ldsum = sb.tile([128, 1], f32)
            nc.scalar.activation(
                logden[:],
                denr[:],
                mybir.ActivationFunctionType.Ln,
                bias=lnbias[:],
                accum_out=ldsum[:],
            )
            rowsum = sb.tile([128, 1], f32)
            nc.vector.tensor_tensor(
                out=rowsum[:],
                in0=ldsum[:],
                in1=possum[:],
                op=mybir.AluOpType.subtract,
            )
            # cross-partition sum via ones-matmul
            pt = ps.tile([128, CGW], f32, name="pt", tag="pt")
            nc.tensor.matmul(pt[:1, :1], ones[:], rowsum[:], start=True, stop=True)
            outsb = sb.tile([1, 1], f32)
            nc.vector.tensor_copy(outsb[:], pt[:1, :1])
            nc.sync.dma_start(out_d[:], outsb[:])

    nc.compile()
    return nc


def _prep_in_maps(z_i: np.ndarray, z_j: np.ndarray) -> list[dict]:
    z = np.concatenate(
        [np.asarray(z_i, dtype=np.float32), np.asarray(z_j, dtype=np.float32)],
        axis=0,
    )
    norms = np.sqrt(np.sum(z * z, axis=1, keepdims=True))
    zn = z / np.maximum(norms, EPS)
    znT = np.ascontiguousarray(zn.T).astype(ml_dtypes.bfloat16)
    znT_k = np.ascontiguousarray(znT.reshape(KCH, 128, N2))
    # fp8 DoubleRow packing: znT8[ki, j, c] = 16 * zn[c, j*128+ki]
    zn8 = (zn.T * FP8_SCALE).reshape(KCH, 128, N2).transpose(1, 0, 2)
    znT8 = np.ascontiguousarray(zn8).astype(ml_dtypes.float8_e4m3)
    in_maps = []
    for c in range(NCORES):
        r0 = c * RPC
        p0 = (r0 + B) % N2
        in_maps.append(
            {
                "znT8": znT8,
                "zrT8": np.ascontiguousarray(znT8[:, :, r0:r0 + RPC]),
                "zrowsT": np.ascontiguousarray(znT_k[:, :, r0:r0 + RPC]),
                "zposT": np.ascontiguousarray(znT_k[:, :, p0:p0 + RPC]),
            }
        )
    return in_maps


def _run(in_maps: list[dict], trace: bool = False):
    global _compiled_nc
    if _compiled_nc is None:
        _compiled_nc = _build_nc()
    from concourse.bass_utils import run_bass_kernel_spmd

    return run_bass_kernel_spmd(
        _compiled_nc, in_maps, core_ids=list(range(NCORES)), trace=trace
    )


def kernel(**inputs: np.ndarray) -> np.ndarray:
    in_maps = _prep_in_maps(inputs["z_i"], inputs["z_j"])
    res = _run(in_maps, trace=False)
    total = 0.0
    for r in res.results:
        total += float(r["out"][0, 0])
    return np.array(total / N2, dtype=np.float32)
